# revision 56
# baseline (speedup 1.0000x reference)
"""Trainium2 Bass kernel for an attention-augmented LSTM (CaptioningRNN).

Reference computation (per batch n, T timesteps):
    A_flat = A.reshape(N, H, 16); h0 = c0 = A_flat.mean(-1)
    scores_t = (h_{t-1} @ A_flat) / sqrt(H); w = softmax(scores)
    attn_t = A_flat @ w
    a = x_t @ Wx + h_{t-1} @ Wh + attn_t @ Wattn + b
    i, f, o, g = split(a, 4); c_t = sig(f)*c + sig(i)*tanh(g); h_t = sig(o)*tanh(c_t)

Strategy: data-parallel over batch across 8 cores (32 batch rows each).
Per core:
  Phase A: U = x @ Wx + b precomputed for all timesteps (bf16 weights,
           rows t-major, 2-way PE column tiling to hide LDWEIGHTS) and
           staged to DRAM in bf16.
  Phase B: 64 recurrent steps. The gate matmul contracts [h; attn]
           (2048-dim, bf16) against W2 = [Wh; Wattn] with gate-interleaved
           columns. Four PE column tiles (tile_position=(0,32j)) run
           CONCURRENTLY, one gate block per 32-partition strip, so the
           128x512 PSUM tile is directly the quad-stacked (4 blocks x 32
           batch) gate layout used by the fused LSTM math - no merge ops.
           U enters each strip via an identity-matmul. Attention scores are
           all-pairs matmuls (also 4-way column-tiled, k-chunks striped
           across strips), extracted with a full-128-partition mask+reduce
           plus 3 tiny strip adds. h is transposed back to [h-dim, batch]
           with a PE-mode transpose (not DMA).

Weight-matrix column order (gate interleave): block j (512 cols) holds
original columns [i|f|o|g][j*128:(j+1)*128]. The same permutation is applied
to Wx, b and hence U.
"""

import math
import os

import numpy as np
import ml_dtypes

import concourse.bass as bass
import concourse.mybir as mybir
import concourse.tile as tile
from concourse import bacc

N, T, D, H = 256, 64, 1024, 1024
NCORES = 8
NB = N // NCORES          # 32 batch rows per core
G = 4 * H                 # 4096 gate columns
P = 16                    # attention positions (4x4)
KH = H // 128             # 8 contraction chunks for h
K2 = (2 * H) // 128       # 16 contraction chunks for [h; attn]
GB = G // 512             # 8 gate blocks of 512
F32 = mybir.dt.float32
BF16 = mybir.dt.bfloat16
BF = ml_dtypes.bfloat16

AF = mybir.ActivationFunctionType
ALU = mybir.AluOpType
AXX = mybir.AxisListType.X

_NC_CACHE = {}


def _gate_perm():
    """perm[new_col] = old_col for the gate-interleaved layout."""
    perm = np.empty(G, dtype=np.int64)
    for j in range(GB):
        for s in range(4):  # i, f, o, g
            perm[j * 512 + s * 128:(j * 512 + (s + 1) * 128)] = np.arange(
                s * H + j * 128, s * H + (j + 1) * 128)
    return perm


def build_nc(t_steps=T):
    """Build the SPMD Bass program (identical on all cores)."""
    nc = bacc.Bacc("TRN2", target_bir_lowering=False, debug=False,
                   num_devices=NCORES)

    xT_d = nc.dram_tensor("xT", [D, t_steps * NB], BF16, kind="ExternalInput")
    wx_d = nc.dram_tensor("wx", [D, G], BF16, kind="ExternalInput")
    w2_d = nc.dram_tensor("w2", [2 * H, G], BF16, kind="ExternalInput")
    b128_d = nc.dram_tensor("b128", [128, G], F32, kind="ExternalInput")
    at_d = nc.dram_tensor("at", [H, NB * P], BF16, kind="ExternalInput")
    h0t2_d = nc.dram_tensor("h0t2", [2 * 128, 128], BF16, kind="ExternalInput")
    h0q_d = nc.dram_tensor("h0q", [2 * 128, 128], F32, kind="ExternalInput")
    mask_d = nc.dram_tensor("mask", [128, NB * P], BF16, kind="ExternalInput")
    ident_d = nc.dram_tensor("ident", [NB, NB], BF16, kind="ExternalInput")
    id128_d = nc.dram_tensor("id128", [128, 128], BF16, kind="ExternalInput")
    ones_d = nc.dram_tensor("ones", [1, 128], BF16, kind="ExternalInput")
    sum4_d = nc.dram_tensor("sum4", [128, NB], F32, kind="ExternalInput")
    out_d = nc.dram_tensor("out", [NB, t_steps, H], BF16,
                           kind="ExternalOutput")

    n_row_tiles = (t_steps * NB) // 128

    with tile.TileContext(nc) as tc:
        with tc.tile_pool(name="dram", bufs=1, space="DRAM") as dpool:
            u_dram = dpool.tile([t_steps * NB, G], BF16)

            # Wh half of W2 prefetches during phase A (SBUF budget allows
            # only half; the Wattn half loads at phase B start and is not
            # needed until step 0's attn rounds).
            wh_pool_cm = tc.tile_pool(name="wh", bufs=1)
            wh_pool = wh_pool_cm.__enter__()
            w2 = []
            for k in range(KH):
                t_ = wh_pool.tile([128, G], BF16, tag=f"w2_{k}")
                nc.sync.dma_start(t_[:], w2_d[k * 128:(k + 1) * 128, :])
                w2.append(t_)

            # ---------------- Phase A: U = x @ Wx + b ----------------
            # 2-way PE column tiling (M=64 halves) so each half's
            # LDWEIGHTS hides under the other half's 512-col stream.
            with tc.tile_pool(name="pa_res", bufs=1) as pa, \
                 tc.tile_pool(name="pa_ps", bufs=4, space="PSUM") as pa_ps, \
                 tc.tile_pool(name="pa_sb", bufs=6) as pa_sb:
                xT = []
                for d in range(KH):
                    t_ = pa.tile([128, t_steps * NB], BF16, tag=f"xT{d}")
                    nc.sync.dma_start(t_[:], xT_d[d * 128:(d + 1) * 128, :])
                    xT.append(t_)
                wx = []
                for d in range(KH):
                    t_ = pa.tile([128, G], BF16, tag=f"wx{d}")
                    nc.sync.dma_start(t_[:], wx_d[d * 128:(d + 1) * 128, :])
                    wx.append(t_)
                b128 = pa.tile([128, G], F32, tag="b128")
                nc.sync.dma_start(b128[:], b128_d[:])

                for m in range(n_row_tiles):
                    for g in range(GB):
                        gs = slice(g * 512, (g + 1) * 512)
                        ps = pa_ps.tile([128, 512], F32, tag="ps")
                        for d in range(KH):
                            for j in range(2):
                                ms = slice(m * 128 + 64 * j,
                                           m * 128 + 64 * j + 64)
                                nc.tensor.matmul(
                                    ps[64 * j:64 * j + 64, :],
                                    xT[d][:, ms], wx[d][:, gs],
                                    start=(d == 0), stop=(d == KH - 1),
                                    tile_position=(0, 64 * j),
                                    skip_group_check=True)
                        us = pa_sb.tile([128, 512], BF16, tag="us")
                        nc.vector.tensor_add(us[:], ps[:], b128[:, gs])
                        ms_full = slice(m * 128, (m + 1) * 128)
                        nc.sync.dma_start(u_dram[ms_full, gs], us[:])

            # ---------------- Phase B: recurrence ----------------
            with tc.tile_pool(name="res", bufs=1) as res, \
                 tc.tile_pool(name="ht", bufs=3) as htp, \
                 tc.tile_pool(name="u", bufs=2) as up, \
                 tc.tile_pool(name="st", bufs=2) as stp, \
                 tc.tile_pool(name="att", bufs=2) as attp, \
                 tc.tile_pool(name="abt", bufs=10) as abtp, \
                 tc.tile_pool(name="blk", bufs=2) as blkp, \
                 tc.tile_pool(name="psg", bufs=3, space="PSUM") as psg_p, \
                 tc.tile_pool(name="pss", bufs=1, space="PSUM") as pss_p, \
                 tc.tile_pool(name="pst", bufs=1, space="PSUM") as pst_p, \
                 tc.tile_pool(name="psw", bufs=1, space="PSUM") as psw_p, \
                 tc.tile_pool(name="psc", bufs=1, space="PSUM") as psc_p:

                for k in range(KH, K2):
                    t_ = res.tile([128, G], BF16, tag=f"w2_{k}")
                    nc.sync.dma_start(t_[:], w2_d[k * 128:(k + 1) * 128, :])
                    w2.append(t_)
                at_all = res.tile([128, KH * NB * P], BF16, tag="at_all")
                for k in range(KH):
                    nc.sync.dma_start(
                        at_all[:, k * NB * P:(k + 1) * NB * P],
                        at_d[k * 128:(k + 1) * 128, :])
                at = [at_all[:, k * NB * P:(k + 1) * NB * P]
                      for k in range(KH)]
                mask = res.tile([128, NB * P], BF16, tag="mask")
                nc.sync.dma_start(mask[:], mask_d[:])
                ident = res.tile([NB, NB], BF16, tag="ident")
                nc.sync.dma_start(ident[:], ident_d[:])
                id128 = res.tile([128, 128], BF16, tag="id128")
                nc.sync.dma_start(id128[:], id128_d[:])
                ones = res.tile([1, 128], BF16, tag="ones")
                nc.sync.dma_start(ones[:], ones_d[:])
                sum4 = res.tile([128, NB], F32, tag="sum4")
                nc.sync.dma_start(sum4[:], sum4_d[:])

                # hT as two packed [128, 128] tiles: tile q column 32*g+n
                # holds h[n, 128*(4q+g) + c] for partition c.
                hTq = []
                for q in range(2):
                    t_ = htp.tile([128, 128], BF16, tag="htq",
                                  name=f"h0t{q}")
                    nc.sync.dma_start(t_[:], h0t2_d[q * 128:(q + 1) * 128, :])
                    hTq.append(t_)

                def ht_chunk(k):
                    q, gp = divmod(k, 4)
                    return hTq[q][:, 32 * gp:32 * gp + 32]

                c_b = []
                for q in range(2):
                    t_ = blkp.tile([128, 128], F32, tag="c", bufs=4,
                                   name=f"c0_{q}")
                    nc.sync.dma_start(t_[:], h0q_d[q * 128:(q + 1) * 128, :])
                    c_b.append(t_)

                u_t = up.tile([NB, G], BF16, tag="u")
                nc.sync.dma_start(u_t[:], u_dram[0:NB, :])

                inv_sqrt_h = 1.0 / math.sqrt(H)

                def gmm(pg, j, lhs, rhs, start, stop):
                    """One gate matmul into strip j (col tile (0, 32j))."""
                    nc.tensor.matmul(pg[32 * j:32 * j + NB, :], lhs, rhs,
                                     start=start, stop=stop,
                                     tile_position=(0, 32 * j),
                                     skip_group_check=True)

                def umm(pg, j, u, gsl):
                    """Init strip j of pg with the U slice (ident matmul)."""
                    nc.tensor.matmul(pg[32 * j:32 * j + NB, :], ident[:],
                                     u[:, gsl], start=True, stop=False,
                                     tile_position=(0, 32 * j),
                                     skip_group_check=True)

                def smm(ps_s, j, k, hts, start):
                    """Score partial for k-chunk k into strip j."""
                    nc.tensor.matmul(ps_s[32 * j:32 * j + NB, :], hts, at[k],
                                     start=start, stop=False,
                                     tile_position=(0, 32 * j),
                                     skip_group_check=True)

                def new_gate_psums(t):
                    return [psg_p.tile([128, 512], F32, tag="g",
                                       name=f"pg{q}_{t}") for q in range(2)]

                def issue_umm(psg, u):
                    for q in range(2):
                        for j in range(4):
                            g = 4 * q + j
                            umm(psg[q], j, u, slice(g * 512, (g + 1) * 512))

                def issue_h_part(psg, hts, ks):
                    # rounds: all 4 strips of a quad run concurrently
                    for k in ks:
                        for q in range(2):
                            for j in range(4):
                                g = 4 * q + j
                                gsl = slice(g * 512, (g + 1) * 512)
                                gmm(psg[q], j, hts(k), w2[k][:, gsl],
                                    False, False)

                def issue_scores(ps_s, hts, ks):
                    for k in ks:
                        smm(ps_s, k % 4, k, hts(k), start=(k < 4))

                def issue_masked(ps_s):
                    """Masked diag extraction of the striped all-pairs
                    scores: [128, 16] per-strip partials in (p) layout."""
                    masked = stp.tile([128, NB * P], F32, tag="masked")
                    nc.vector.tensor_tensor(
                        out=masked[:].rearrange("m (p n) -> m p n", n=NB),
                        in0=ps_s[:].rearrange("m (n p) -> m p n", p=P),
                        in1=mask[:].rearrange("m (n p) -> m p n", p=P),
                        op=ALU.mult)
                    sc4 = stp.tile([128, P], F32, tag="sc4")
                    nc.vector.tensor_reduce(
                        sc4[:], masked[:].rearrange("m (p n) -> m p n", n=NB),
                        axis=AXX, op=ALU.add)
                    return sc4

                def issue_bcast(sc4, t):
                    """Strip-sum via stacked-identity fp32 matmul -> [NB, P]
                    scores in PSUM."""
                    scps = psc_p.tile([NB, P], F32, tag="scps",
                                      name=f"scps{t}")
                    nc.tensor.matmul(scps[:], sum4[:], sc4[:],
                                     start=True, stop=True)
                    return scps

                # ---- prologue: scores S_0, U_0 + h-part of all strips ----
                ps_s = pss_p.tile([128, NB * P], F32, tag="s", name="s0")
                issue_scores(ps_s, ht_chunk, range(KH))
                psg = new_gate_psums(0)
                issue_umm(psg, u_t)
                issue_h_part(psg, ht_chunk, range(KH))
                sc = issue_bcast(issue_masked(ps_s), 0)

                def quad_math(q, psg, c_b, c_new, hbf, t):
                    """Fused LSTM gate math on the quad-stacked psum tile.
                    ACT does the nonlinearities. quad0's multiplies run on
                    GpSimd (DVE is still pooling then); quad1's run on DVE
                    (pooling done -> faster step tail)."""
                    eng = nc.gpsimd if q == 0 else nc.vector
                    gq = psg[q]
                    sio = blkp.tile([128, 384], F32, tag="sio")
                    nc.scalar.activation(sio[:], gq[:, 0:384], AF.Sigmoid)
                    tg = blkp.tile([128, 128], F32, tag="tg")
                    nc.scalar.activation(tg[:], gq[:, 384:512], AF.Tanh)
                    m1 = blkp.tile([128, 128], F32, tag="m1")
                    eng.tensor_tensor(out=m1[:], in0=sio[:, 0:128],
                                      in1=tg[:], op=ALU.mult)
                    m2 = blkp.tile([128, 128], F32, tag="m2")
                    eng.tensor_tensor(out=m2[:], in0=sio[:, 128:256],
                                      in1=c_b[q][:], op=ALU.mult)
                    eng.tensor_add(c_new[q][:], m1[:], m2[:])
                    tcn = blkp.tile([128, 128], F32, tag="tcn")
                    nc.scalar.activation(tcn[:], c_new[q][:], AF.Tanh)
                    eng.tensor_tensor(out=hbf[q][:],
                                      in0=sio[:, 256:384],
                                      in1=tcn[:], op=ALU.mult)
                    qsl = slice(q * 512, (q + 1) * 512)
                    nc.sync.dma_start(
                        out_d[:, t, qsl].rearrange("n (g c) -> g n c", g=4),
                        hbf[q][:])

                for t in range(t_steps):
                    last = (t + 1 >= t_steps)
                    if not last:
                        u_next = up.tile([NB, G], BF16, tag="u")
                        nc.scalar.dma_start(
                            u_next[:], u_dram[(t + 1) * NB:(t + 2) * NB, :])

                    # (a) small-domain softmax on the [NB, P] scores psum
                    sm_sc = nc.enter_named_scope(f"sm{t}", False)
                    # exp(x) = s/(1-s) with s = sigmoid(x): keeps the ACT
                    # table at {Sigmoid, Tanh} with no per-step reloads
                    sg = stp.tile([NB, P], F32, tag="sg")
                    nc.scalar.activation(sg[:], sc[:], AF.Sigmoid,
                                         scale=float(inv_sqrt_h))
                    om = stp.tile([NB, P], F32, tag="om")
                    nc.scalar.activation(om[:], sc[:], AF.Sigmoid,
                                         scale=float(-inv_sqrt_h))
                    omr = stp.tile([NB, P], F32, tag="omr")
                    nc.vector.reciprocal(omr[:], om[:])
                    expw = stp.tile([NB, P], F32, tag="expw")
                    nc.vector.tensor_tensor(out=expw[:], in0=sg[:],
                                            in1=omr[:], op=ALU.mult)
                    sume = stp.tile([NB, 1], F32, tag="sume")
                    nc.vector.tensor_reduce(sume[:], expw[:], axis=AXX,
                                            op=ALU.add)
                    rec = stp.tile([NB, 1], F32, tag="rec")
                    nc.vector.reciprocal(rec[:], sume[:])
                    w16 = stp.tile([NB, P], BF16, tag="w16")
                    nc.vector.tensor_scalar(out=w16[:], in0=expw[:],
                                            scalar1=rec[:], scalar2=None,
                                            op0=ALU.mult)
                    # flatten [NB, P] -> [1, NB*P]: SBUF->SBUF DMA gather
                    w1 = stp.tile([1, NB * P], BF16, tag="w1")
                    nc.scalar.dma_start(w1[:], w16[:])
                    # broadcast to 128 partitions via ones-matmul, then the
                    # two packed copies run on ACT and DVE in parallel
                    ps_w = psw_p.tile([128, NB * P], F32, tag="w",
                                      name=f"w{t}")
                    nc.tensor.matmul(ps_w[:], ones[:], w1[:],
                                     start=True, stop=True)
                    wfull = attp.tile([128, NB * P], BF16, tag="wfull")
                    nc.scalar.activation(wfull[:], ps_w[:], AF.Copy)
                    # HAM warm-keepers: tiny matmuls whose deps fire mid-hole,
                    # so the PE never sees a full idle window and the clock
                    # gate stays at 2.4 GHz
                    scr = psc_p.tile([NB, 512], F32, tag="scr",
                                     name=f"scr{t}")
                    for dep in (omr, expw):
                        nc.tensor.matmul(scr[:, 0:P], sum4[0:NB, :], dep[:],
                                         start=True, stop=True,
                                         skip_group_check=True)
                    nc.tensor.matmul(scr[:, 0:P], ident[:], w16[:],
                                     start=True, stop=True,
                                     skip_group_check=True)
                    nc.tensor.matmul(scr[:], ident[:], wfull[0:NB, :],
                                     start=True, stop=True,
                                     skip_group_check=True)
                    nc.leave_named_scope(f"sm{t}", sm_sc[0], False)

                    # (b) attention pooling -> attnT: contiguous 2D multiply
                    # + 3D-view reduce pairs over the packed AT tile
                    sc_ = nc.enter_named_scope(f"att{t}", False)
                    attnT = []
                    with nc.allow_low_precision("attn pooled in bf16 anyway"):
                        for h in range(4):
                            hs = slice(h * 2 * NB * P, (h + 1) * 2 * NB * P)
                            # bufs=1 forces pr(h+1) to wait for t8(h), so the
                            # scheduler cannot delay pair h's sum tree behind
                            # the next pair's multiply
                            pr = attp.tile([128, 2 * NB * P], BF16, tag="pr",
                                           bufs=1)
                            nc.vector.tensor_tensor(
                                out=pr[:].rearrange("m (k x) -> m k x", k=2),
                                in0=at_all[:, hs].rearrange(
                                    "m (k x) -> m k x", k=2),
                                in1=bass.AP(wfull[:].tensor, wfull[:].offset,
                                            [wfull[:].ap[0], [0, 2],
                                             wfull[:].ap[1]]),
                                op=ALU.mult)
                            # p-sum as a strided add tree: every level keeps
                            # unit innermost stride, staying in DVE 2x mode
                            # (tensor_reduce only runs 1x)
                            t8 = attp.tile([128, 2 * NB * 8], BF16, tag="t8")
                            nc.vector.tensor_tensor(
                                out=t8[:].rearrange("m (y p) -> m y p", p=8),
                                in0=pr[:].rearrange("m (y p) -> m y p",
                                                    p=P)[:, :, 0:8],
                                in1=pr[:].rearrange("m (y p) -> m y p",
                                                    p=P)[:, :, 8:16],
                                op=ALU.add)
                            if h < 2:
                                # large warm-keeper mid-pooling: sustains PE
                                # activity so HAM ramps to 2.4 GHz before the
                                # attn rounds begin
                                nc.tensor.matmul(
                                    scr[:], ident[:], t8[0:NB, 0:512],
                                    start=True, stop=True,
                                    skip_group_check=True)
                            t4 = attp.tile([128, 2 * NB * 4], BF16, tag="t4")
                            nc.vector.tensor_tensor(
                                out=t4[:].rearrange("m (y p) -> m y p", p=4),
                                in0=t8[:].rearrange("m (y p) -> m y p",
                                                    p=8)[:, :, 0:4],
                                in1=t8[:].rearrange("m (y p) -> m y p",
                                                    p=8)[:, :, 4:8],
                                op=ALU.add)
                            t2 = attp.tile([128, 2 * NB * 2], BF16, tag="t2")
                            nc.vector.tensor_tensor(
                                out=t2[:].rearrange("m (y p) -> m y p", p=2),
                                in0=t4[:].rearrange("m (y p) -> m y p",
                                                    p=4)[:, :, 0:2],
                                in1=t4[:].rearrange("m (y p) -> m y p",
                                                    p=4)[:, :, 2:4],
                                op=ALU.add)
                            ab4 = abtp.tile([128, 2 * NB], BF16, tag="ab")
                            nc.vector.tensor_tensor(
                                out=ab4[:].rearrange("m (y p) -> m y p", p=1),
                                in0=t2[:].rearrange("m (y p) -> m y p",
                                                    p=2)[:, :, 0:1],
                                in1=t2[:].rearrange("m (y p) -> m y p",
                                                    p=2)[:, :, 1:2],
                                op=ALU.add)
                            for kk in range(2):
                                attnT.append(
                                    ab4[:, kk * NB:(kk + 1) * NB])
                    nc.leave_named_scope(f"att{t}", sc_[0], False)

                    # (c) attn-part matmuls per quad: quad0 fully first, so
                    # quad0's LSTM math overlaps quad1's matmul rounds
                    sc2 = nc.enter_named_scope(f"ga{t}", False)
                    c_new = [blkp.tile([128, 128], F32, tag="c", bufs=4,
                                       name=f"cn{q}_{t}") for q in range(2)]
                    hbf = [blkp.tile([128, 128], BF16, tag="hbf",
                                     name=f"hbf{q}_{t}") for q in range(2)]
                    for q in range(2):
                        for k in range(KH, K2):
                            for j in range(4):
                                g = 4 * q + j
                                gsl = slice(g * 512, (g + 1) * 512)
                                gmm(psg[q], j, attnT[k - KH], w2[k][:, gsl],
                                    False, k == K2 - 1)
                        quad_math(q, psg, c_b, c_new, hbf, t)
                    nc.leave_named_scope(f"ga{t}", sc2[0], False)

                    # (d) next step: transpose each quad's h as it lands,
                    # U-init filler between, then scores + h-part split by
                    # which hT tile they need
                    sc4_ = nc.enter_named_scope(f"nxt{t}", False)
                    if not last:
                        psg2 = new_gate_psums(t + 1)
                        tr_ps = pst_p.tile([128, 256], BF16, tag="tr",
                                           name=f"tr{t}")
                        hT_new = []

                        def tr_quad(q):
                            nc.tensor.transpose(
                                tr_ps[:, q * 128:(q + 1) * 128],
                                hbf[q][:], id128[:])
                            ht_ = htp.tile([128, 128], BF16, tag="htq",
                                           name=f"ht{q}_{t}")
                            nc.vector.tensor_copy(
                                ht_[:], tr_ps[:, q * 128:(q + 1) * 128])
                            hT_new.append(ht_)

                        def hts_new(k):
                            q, gp = divmod(k, 4)
                            return hT_new[q][:, 32 * gp:32 * gp + 32]

                        tr_quad(0)
                        issue_umm(psg2, u_next)
                        tr_quad(1)
                        ps_s2 = pss_p.tile([128, NB * P], F32, tag="s",
                                           name=f"s{t + 1}")
                        issue_scores(ps_s2, hts_new, range(4))
                        issue_scores(ps_s2, hts_new, range(4, KH))
                        # masked extraction (DVE) runs while the first h-part
                        # rounds stream; the strip-sum matmul is slotted
                        # after 2 k-rounds so its input is ready on arrival
                        sc4_t = issue_masked(ps_s2)
                        issue_h_part(psg2, hts_new, range(2))
                        sc = issue_bcast(sc4_t, t + 1)
                        issue_h_part(psg2, hts_new, range(2, KH))
                        ps_s = ps_s2
                        psg = psg2
                        hTq = hT_new
                        c_b = c_new
                        u_t = u_next
                    nc.leave_named_scope(f"nxt{t}", sc4_[0], False)

            wh_pool_cm.__exit__(None, None, None)

    nc.compile()
    return nc


def prepare_inputs(x, A, Wx, Wh, Wattn, b, t_steps=T):
    """Host-side sharding + layout prep. Returns list of per-core input maps."""
    x = np.asarray(x, dtype=np.float32)
    A = np.asarray(A, dtype=np.float32)
    Wx = np.asarray(Wx, dtype=np.float32)
    Wh = np.asarray(Wh, dtype=np.float32)
    Wattn = np.asarray(Wattn, dtype=np.float32)
    b = np.asarray(b, dtype=np.float32)

    perm = _gate_perm()
    wx_p = np.ascontiguousarray(Wx[:, perm]).astype(BF)
    w2_p = np.ascontiguousarray(np.vstack([Wh, Wattn])[:, perm]).astype(BF)
    b128 = np.ascontiguousarray(
        np.broadcast_to(b[perm], (128, G))).astype(np.float32)
    mask32 = np.zeros((NB, NB * P), dtype=BF)
    for n in range(NB):
        mask32[n, n * P:(n + 1) * P] = 1
    mask = np.ascontiguousarray(np.tile(mask32, (4, 1)))
    ident = np.eye(NB, dtype=BF)
    id128 = np.eye(128, dtype=BF)
    ones = np.ones((1, 128), dtype=BF)
    sum4 = np.ascontiguousarray(
        np.tile(np.eye(NB, dtype=np.float32), (4, 1)))

    in_maps = []
    for c in range(NCORES):
        x_c = x[c * NB:(c + 1) * NB, :t_steps]          # (NB, t, D)
        xr = x_c.transpose(1, 0, 2).reshape(t_steps * NB, D)  # t-major rows
        xT = np.ascontiguousarray(xr.T).astype(BF)       # (D, t*NB)
        A_c = A[c * NB:(c + 1) * NB].reshape(NB, H, P)
        at_c = np.ascontiguousarray(
            A_c.transpose(1, 0, 2).reshape(H, NB * P)).astype(BF)
        h0 = A_c.mean(axis=2).astype(np.float32)         # (NB, H)
        # quad-stacked initial cell state: block g of quad q lives on
        # partitions 32*(g%4), columns = h dims within the block
        h0q = np.empty((2 * 128, 128), dtype=np.float32)
        for g in range(8):
            q, gp = divmod(g, 4)
            h0q[q * 128 + gp * 32:q * 128 + (gp + 1) * 32, :] = \
                h0[:, g * 128:(g + 1) * 128]
        # packed transposed h0: tile q, col 32*gp+n, row c
        h0t2 = np.ascontiguousarray(
            h0q.reshape(2, 128, 128).transpose(0, 2, 1)
            .reshape(2 * 128, 128)).astype(BF)
        in_maps.append({
            "xT": xT, "wx": wx_p, "w2": w2_p, "b128": b128,
            "at": at_c, "h0t2": h0t2, "h0q": h0q,
            "mask": mask, "ones": ones, "ident": ident, "id128": id128,
            "sum4": sum4,
        })
    return in_maps


def kernel(x, A, Wx, Wh, Wattn, b):
    from concourse.bass_utils import run_bass_kernel_spmd

    key = T
    if key not in _NC_CACHE:
        _NC_CACHE[key] = build_nc(T)
    nc = _NC_CACHE[key]

    in_maps = prepare_inputs(x, A, Wx, Wh, Wattn, b)
    trace = bool(int(os.environ.get("KERNEL_TRACE", "0")))
    res = run_bass_kernel_spmd(nc, in_maps, core_ids=list(range(NCORES)),
                               trace=trace)
    if res.exec_time_ns is not None:
        print(f"HW exec time: {res.exec_time_ns} ns")
        kernel.last_exec_time_ns = res.exec_time_ns
    out = np.concatenate([r["out"] for r in res.results], axis=0)
    return out.astype(np.float32)


kernel.last_exec_time_ns = None


# revision 57
# speedup vs baseline: 1.0015x; 1.0015x over previous
"""Trainium2 Bass kernel for an attention-augmented LSTM (CaptioningRNN).

Reference computation (per batch n, T timesteps):
    A_flat = A.reshape(N, H, 16); h0 = c0 = A_flat.mean(-1)
    scores_t = (h_{t-1} @ A_flat) / sqrt(H); w = softmax(scores)
    attn_t = A_flat @ w
    a = x_t @ Wx + h_{t-1} @ Wh + attn_t @ Wattn + b
    i, f, o, g = split(a, 4); c_t = sig(f)*c + sig(i)*tanh(g); h_t = sig(o)*tanh(c_t)

Strategy: data-parallel over batch across 8 cores (32 batch rows each).
Per core:
  Phase A: U = x @ Wx + b precomputed for all timesteps (bf16 weights,
           rows t-major, 2-way PE column tiling to hide LDWEIGHTS) and
           staged to DRAM in bf16.
  Phase B: 64 recurrent steps. The gate matmul contracts [h; attn]
           (2048-dim, bf16) against W2 = [Wh; Wattn] with gate-interleaved
           columns. Four PE column tiles (tile_position=(0,32j)) run
           CONCURRENTLY, one gate block per 32-partition strip, so the
           128x512 PSUM tile is directly the quad-stacked (4 blocks x 32
           batch) gate layout used by the fused LSTM math - no merge ops.
           U enters each strip via an identity-matmul. Attention scores are
           all-pairs matmuls (also 4-way column-tiled, k-chunks striped
           across strips), extracted with a full-128-partition mask+reduce
           plus 3 tiny strip adds. h is transposed back to [h-dim, batch]
           with a PE-mode transpose (not DMA).

Weight-matrix column order (gate interleave): block j (512 cols) holds
original columns [i|f|o|g][j*128:(j+1)*128]. The same permutation is applied
to Wx, b and hence U.
"""

import math
import os

import numpy as np
import ml_dtypes

import concourse.bass as bass
import concourse.mybir as mybir
import concourse.tile as tile
from concourse import bacc

N, T, D, H = 256, 64, 1024, 1024
NCORES = 8
NB = N // NCORES          # 32 batch rows per core
G = 4 * H                 # 4096 gate columns
P = 16                    # attention positions (4x4)
KH = H // 128             # 8 contraction chunks for h
K2 = (2 * H) // 128       # 16 contraction chunks for [h; attn]
GB = G // 512             # 8 gate blocks of 512
F32 = mybir.dt.float32
BF16 = mybir.dt.bfloat16
BF = ml_dtypes.bfloat16

AF = mybir.ActivationFunctionType
ALU = mybir.AluOpType
AXX = mybir.AxisListType.X

_NC_CACHE = {}


def _gate_perm():
    """perm[new_col] = old_col for the gate-interleaved layout."""
    perm = np.empty(G, dtype=np.int64)
    for j in range(GB):
        for s in range(4):  # i, f, o, g
            perm[j * 512 + s * 128:(j * 512 + (s + 1) * 128)] = np.arange(
                s * H + j * 128, s * H + (j + 1) * 128)
    return perm


def build_nc(t_steps=T):
    """Build the SPMD Bass program (identical on all cores)."""
    nc = bacc.Bacc("TRN2", target_bir_lowering=False, debug=False,
                   num_devices=NCORES)

    xT_d = nc.dram_tensor("xT", [D, t_steps * NB], BF16, kind="ExternalInput")
    wx_d = nc.dram_tensor("wx", [D, G], BF16, kind="ExternalInput")
    w2_d = nc.dram_tensor("w2", [2 * H, G], BF16, kind="ExternalInput")
    b128_d = nc.dram_tensor("b128", [128, G], F32, kind="ExternalInput")
    at_d = nc.dram_tensor("at", [H, NB * P], BF16, kind="ExternalInput")
    h0t2_d = nc.dram_tensor("h0t2", [2 * 128, 128], BF16, kind="ExternalInput")
    h0q_d = nc.dram_tensor("h0q", [2 * 128, 128], F32, kind="ExternalInput")
    mask_d = nc.dram_tensor("mask", [128, NB * P], BF16, kind="ExternalInput")
    ident_d = nc.dram_tensor("ident", [NB, NB], BF16, kind="ExternalInput")
    id128_d = nc.dram_tensor("id128", [128, 128], BF16, kind="ExternalInput")
    ones_d = nc.dram_tensor("ones", [1, 128], BF16, kind="ExternalInput")
    sum4_d = nc.dram_tensor("sum4", [128, NB], F32, kind="ExternalInput")
    out_d = nc.dram_tensor("out", [NB, t_steps, H], BF16,
                           kind="ExternalOutput")

    n_row_tiles = (t_steps * NB) // 128

    with tile.TileContext(nc) as tc:
        with tc.tile_pool(name="dram", bufs=1, space="DRAM") as dpool:
            u_dram = dpool.tile([t_steps * NB, G], BF16)

            # Wh half of W2 prefetches during phase A (SBUF budget allows
            # only half; the Wattn half loads at phase B start and is not
            # needed until step 0's attn rounds).
            wh_pool_cm = tc.tile_pool(name="wh", bufs=1)
            wh_pool = wh_pool_cm.__enter__()
            w2 = []
            for k in range(KH):
                t_ = wh_pool.tile([128, G], BF16, tag=f"w2_{k}")
                nc.sync.dma_start(t_[:], w2_d[k * 128:(k + 1) * 128, :])
                w2.append(t_)

            # ---------------- Phase A: U = x @ Wx + b ----------------
            # 2-way PE column tiling (M=64 halves) so each half's
            # LDWEIGHTS hides under the other half's 512-col stream.
            with tc.tile_pool(name="pa_res", bufs=1) as pa, \
                 tc.tile_pool(name="pa_ps", bufs=4, space="PSUM") as pa_ps, \
                 tc.tile_pool(name="pa_sb", bufs=6) as pa_sb:
                xT = []
                for d in range(KH):
                    t_ = pa.tile([128, t_steps * NB], BF16, tag=f"xT{d}")
                    nc.sync.dma_start(t_[:], xT_d[d * 128:(d + 1) * 128, :])
                    xT.append(t_)
                wx = []
                for d in range(KH):
                    t_ = pa.tile([128, G], BF16, tag=f"wx{d}")
                    nc.sync.dma_start(t_[:], wx_d[d * 128:(d + 1) * 128, :])
                    wx.append(t_)
                b128 = pa.tile([128, G], F32, tag="b128")
                nc.sync.dma_start(b128[:], b128_d[:])

                for m in range(n_row_tiles):
                    for g in range(GB):
                        gs = slice(g * 512, (g + 1) * 512)
                        ps = pa_ps.tile([128, 512], F32, tag="ps")
                        for d in range(KH):
                            for j in range(2):
                                ms = slice(m * 128 + 64 * j,
                                           m * 128 + 64 * j + 64)
                                nc.tensor.matmul(
                                    ps[64 * j:64 * j + 64, :],
                                    xT[d][:, ms], wx[d][:, gs],
                                    start=(d == 0), stop=(d == KH - 1),
                                    tile_position=(0, 64 * j),
                                    skip_group_check=True)
                        us = pa_sb.tile([128, 512], BF16, tag="us")
                        nc.vector.tensor_add(us[:], ps[:], b128[:, gs])
                        ms_full = slice(m * 128, (m + 1) * 128)
                        nc.sync.dma_start(u_dram[ms_full, gs], us[:])

            # ---------------- Phase B: recurrence ----------------
            with tc.tile_pool(name="res", bufs=1) as res, \
                 tc.tile_pool(name="ht", bufs=3) as htp, \
                 tc.tile_pool(name="u", bufs=2) as up, \
                 tc.tile_pool(name="st", bufs=2) as stp, \
                 tc.tile_pool(name="att", bufs=2) as attp, \
                 tc.tile_pool(name="abt", bufs=10) as abtp, \
                 tc.tile_pool(name="blk", bufs=2) as blkp, \
                 tc.tile_pool(name="psg", bufs=3, space="PSUM") as psg_p, \
                 tc.tile_pool(name="pss", bufs=1, space="PSUM") as pss_p, \
                 tc.tile_pool(name="pst", bufs=1, space="PSUM") as pst_p, \
                 tc.tile_pool(name="psw", bufs=1, space="PSUM") as psw_p, \
                 tc.tile_pool(name="psc", bufs=1, space="PSUM") as psc_p:

                for k in range(KH, K2):
                    t_ = res.tile([128, G], BF16, tag=f"w2_{k}")
                    nc.sync.dma_start(t_[:], w2_d[k * 128:(k + 1) * 128, :])
                    w2.append(t_)
                at_all = res.tile([128, KH * NB * P], BF16, tag="at_all")
                for k in range(KH):
                    nc.sync.dma_start(
                        at_all[:, k * NB * P:(k + 1) * NB * P],
                        at_d[k * 128:(k + 1) * 128, :])
                at = [at_all[:, k * NB * P:(k + 1) * NB * P]
                      for k in range(KH)]
                mask = res.tile([128, NB * P], BF16, tag="mask")
                nc.sync.dma_start(mask[:], mask_d[:])
                ident = res.tile([NB, NB], BF16, tag="ident")
                nc.sync.dma_start(ident[:], ident_d[:])
                id128 = res.tile([128, 128], BF16, tag="id128")
                nc.sync.dma_start(id128[:], id128_d[:])
                ones = res.tile([1, 128], BF16, tag="ones")
                nc.sync.dma_start(ones[:], ones_d[:])
                sum4 = res.tile([128, NB], F32, tag="sum4")
                nc.sync.dma_start(sum4[:], sum4_d[:])

                # hT as two packed [128, 128] tiles: tile q column 32*g+n
                # holds h[n, 128*(4q+g) + c] for partition c.
                hTq = []
                for q in range(2):
                    t_ = htp.tile([128, 128], BF16, tag="htq",
                                  name=f"h0t{q}")
                    nc.sync.dma_start(t_[:], h0t2_d[q * 128:(q + 1) * 128, :])
                    hTq.append(t_)

                def ht_chunk(k):
                    q, gp = divmod(k, 4)
                    return hTq[q][:, 32 * gp:32 * gp + 32]

                c_b = []
                for q in range(2):
                    t_ = blkp.tile([128, 128], F32, tag="c", bufs=4,
                                   name=f"c0_{q}")
                    nc.sync.dma_start(t_[:], h0q_d[q * 128:(q + 1) * 128, :])
                    c_b.append(t_)

                u_t = up.tile([NB, G], BF16, tag="u")
                nc.sync.dma_start(u_t[:], u_dram[0:NB, :])

                inv_sqrt_h = 1.0 / math.sqrt(H)

                def gmm(pg, j, lhs, rhs, start, stop):
                    """One gate matmul into strip j (col tile (0, 32j))."""
                    nc.tensor.matmul(pg[32 * j:32 * j + NB, :], lhs, rhs,
                                     start=start, stop=stop,
                                     tile_position=(0, 32 * j),
                                     skip_group_check=True)

                def umm(pg, j, u, gsl):
                    """Init strip j of pg with the U slice (ident matmul)."""
                    nc.tensor.matmul(pg[32 * j:32 * j + NB, :], ident[:],
                                     u[:, gsl], start=True, stop=False,
                                     tile_position=(0, 32 * j),
                                     skip_group_check=True)

                def smm(ps_s, j, k, hts, start):
                    """Score partial for k-chunk k into strip j."""
                    nc.tensor.matmul(ps_s[32 * j:32 * j + NB, :], hts, at[k],
                                     start=start, stop=False,
                                     tile_position=(0, 32 * j),
                                     skip_group_check=True)

                def new_gate_psums(t):
                    return [psg_p.tile([128, 512], F32, tag="g",
                                       name=f"pg{q}_{t}") for q in range(2)]

                def issue_umm(psg, u):
                    for q in range(2):
                        for j in range(4):
                            g = 4 * q + j
                            umm(psg[q], j, u, slice(g * 512, (g + 1) * 512))

                def issue_h_part(psg, hts, ks):
                    # rounds: all 4 strips of a quad run concurrently
                    for k in ks:
                        for q in range(2):
                            for j in range(4):
                                g = 4 * q + j
                                gsl = slice(g * 512, (g + 1) * 512)
                                gmm(psg[q], j, hts(k), w2[k][:, gsl],
                                    False, False)

                def issue_scores(ps_s, hts, ks):
                    for k in ks:
                        smm(ps_s, k % 4, k, hts(k), start=(k < 4))

                def issue_masked(ps_s):
                    """Masked diag extraction of the striped all-pairs
                    scores: [128, 16] per-strip partials in (p) layout."""
                    masked = stp.tile([128, NB * P], F32, tag="masked")
                    nc.vector.tensor_tensor(
                        out=masked[:].rearrange("m (p n) -> m p n", n=NB),
                        in0=ps_s[:].rearrange("m (n p) -> m p n", p=P),
                        in1=mask[:].rearrange("m (n p) -> m p n", p=P),
                        op=ALU.mult)
                    sc4 = stp.tile([128, P], F32, tag="sc4")
                    nc.vector.tensor_reduce(
                        sc4[:], masked[:].rearrange("m (p n) -> m p n", n=NB),
                        axis=AXX, op=ALU.add)
                    return sc4

                def issue_bcast(sc4, t):
                    """Strip-sum via stacked-identity fp32 matmul -> [NB, P]
                    scores in PSUM."""
                    scps = psc_p.tile([NB, P], F32, tag="scps",
                                      name=f"scps{t}")
                    nc.tensor.matmul(scps[:], sum4[:], sc4[:],
                                     start=True, stop=True)
                    return scps

                # ---- prologue: scores S_0, U_0 + h-part of all strips ----
                ps_s = pss_p.tile([128, NB * P], F32, tag="s", name="s0")
                issue_scores(ps_s, ht_chunk, range(KH))
                psg = new_gate_psums(0)
                issue_umm(psg, u_t)
                issue_h_part(psg, ht_chunk, range(KH))
                sc = issue_bcast(issue_masked(ps_s), 0)

                def quad_math(q, psg, c_b, c_new, hbf, t):
                    """Fused LSTM gate math on the quad-stacked psum tile.
                    ACT does the nonlinearities. quad0's multiplies run on
                    GpSimd (DVE is still pooling then); quad1's run on DVE
                    (pooling done -> faster step tail)."""
                    eng = nc.gpsimd if q == 0 else nc.vector
                    gq = psg[q]
                    sio = blkp.tile([128, 384], F32, tag="sio")
                    nc.scalar.activation(sio[:], gq[:, 0:384], AF.Sigmoid)
                    tg = blkp.tile([128, 128], F32, tag="tg")
                    nc.scalar.activation(tg[:], gq[:, 384:512], AF.Tanh)
                    m1 = blkp.tile([128, 128], F32, tag="m1")
                    eng.tensor_tensor(out=m1[:], in0=sio[:, 0:128],
                                      in1=tg[:], op=ALU.mult)
                    m2 = blkp.tile([128, 128], F32, tag="m2")
                    eng.tensor_tensor(out=m2[:], in0=sio[:, 128:256],
                                      in1=c_b[q][:], op=ALU.mult)
                    eng.tensor_add(c_new[q][:], m1[:], m2[:])
                    tcn = blkp.tile([128, 128], F32, tag="tcn")
                    nc.scalar.activation(tcn[:], c_new[q][:], AF.Tanh)
                    eng.tensor_tensor(out=hbf[q][:],
                                      in0=sio[:, 256:384],
                                      in1=tcn[:], op=ALU.mult)
                    qsl = slice(q * 512, (q + 1) * 512)
                    nc.sync.dma_start(
                        out_d[:, t, qsl].rearrange("n (g c) -> g n c", g=4),
                        hbf[q][:])

                for t in range(t_steps):
                    last = (t + 1 >= t_steps)
                    if not last:
                        u_next = up.tile([NB, G], BF16, tag="u")
                        nc.scalar.dma_start(
                            u_next[:], u_dram[(t + 1) * NB:(t + 2) * NB, :])

                    # (a) small-domain softmax on the [NB, P] scores psum
                    sm_sc = nc.enter_named_scope(f"sm{t}", False)
                    # exp(x) = s/(1-s) with s = sigmoid(x): keeps the ACT
                    # table at {Sigmoid, Tanh} with no per-step reloads
                    sg = stp.tile([NB, P], F32, tag="sg")
                    nc.scalar.activation(sg[:], sc[:], AF.Sigmoid,
                                         scale=float(inv_sqrt_h))
                    om = stp.tile([NB, P], F32, tag="om")
                    nc.scalar.activation(om[:], sc[:], AF.Sigmoid,
                                         scale=float(-inv_sqrt_h))
                    omr = stp.tile([NB, P], F32, tag="omr")
                    nc.vector.reciprocal(omr[:], om[:])
                    expw = stp.tile([NB, P], F32, tag="expw")
                    nc.vector.tensor_tensor(out=expw[:], in0=sg[:],
                                            in1=omr[:], op=ALU.mult)
                    sume = stp.tile([NB, 1], F32, tag="sume")
                    nc.vector.tensor_reduce(sume[:], expw[:], axis=AXX,
                                            op=ALU.add)
                    rec = stp.tile([NB, 1], F32, tag="rec")
                    nc.vector.reciprocal(rec[:], sume[:])
                    w16 = stp.tile([NB, P], BF16, tag="w16")
                    nc.vector.tensor_scalar(out=w16[:], in0=expw[:],
                                            scalar1=rec[:], scalar2=None,
                                            op0=ALU.mult)
                    # flatten [NB, P] -> [1, NB*P]: SBUF->SBUF DMA gather
                    w1 = stp.tile([1, NB * P], BF16, tag="w1")
                    nc.scalar.dma_start(w1[:], w16[:])
                    # broadcast to 128 partitions via ones-matmul, then the
                    # two packed copies run on ACT and DVE in parallel
                    ps_w = psw_p.tile([128, NB * P], F32, tag="w",
                                      name=f"w{t}")
                    nc.tensor.matmul(ps_w[:], ones[:], w1[:],
                                     start=True, stop=True)
                    wfull = attp.tile([128, NB * P], BF16, tag="wfull")
                    nc.scalar.activation(wfull[:], ps_w[:], AF.Copy)
                    # HAM warm-keepers: tiny matmuls whose deps fire mid-hole,
                    # so the PE never sees a full idle window and the clock
                    # gate stays at 2.4 GHz
                    scr = psc_p.tile([NB, 128], F32, tag="scr",
                                     name=f"scr{t}")
                    for dep in (omr, expw):
                        nc.tensor.matmul(scr[:, 0:P], sum4[0:NB, :], dep[:],
                                         start=True, stop=True,
                                         skip_group_check=True)
                    nc.tensor.matmul(scr[:, 0:P], ident[:], w16[:],
                                     start=True, stop=True,
                                     skip_group_check=True)
                    nc.tensor.matmul(scr[:, 0:128], ident[:],
                                     wfull[0:NB, 0:128],
                                     start=True, stop=True,
                                     skip_group_check=True)
                    nc.leave_named_scope(f"sm{t}", sm_sc[0], False)

                    # (b) attention pooling -> attnT: contiguous 2D multiply
                    # + 3D-view reduce pairs over the packed AT tile
                    sc_ = nc.enter_named_scope(f"att{t}", False)
                    attnT = []
                    with nc.allow_low_precision("attn pooled in bf16 anyway"):
                        for h in range(4):
                            hs = slice(h * 2 * NB * P, (h + 1) * 2 * NB * P)
                            # bufs=1 forces pr(h+1) to wait for t8(h), so the
                            # scheduler cannot delay pair h's sum tree behind
                            # the next pair's multiply
                            pr = attp.tile([128, 2 * NB * P], BF16, tag="pr",
                                           bufs=1)
                            nc.vector.tensor_tensor(
                                out=pr[:].rearrange("m (k x) -> m k x", k=2),
                                in0=at_all[:, hs].rearrange(
                                    "m (k x) -> m k x", k=2),
                                in1=bass.AP(wfull[:].tensor, wfull[:].offset,
                                            [wfull[:].ap[0], [0, 2],
                                             wfull[:].ap[1]]),
                                op=ALU.mult)
                            # p-sum as a strided add tree: every level keeps
                            # unit innermost stride, staying in DVE 2x mode
                            # (tensor_reduce only runs 1x)
                            t8 = attp.tile([128, 2 * NB * 8], BF16, tag="t8")
                            nc.vector.tensor_tensor(
                                out=t8[:].rearrange("m (y p) -> m y p", p=8),
                                in0=pr[:].rearrange("m (y p) -> m y p",
                                                    p=P)[:, :, 0:8],
                                in1=pr[:].rearrange("m (y p) -> m y p",
                                                    p=P)[:, :, 8:16],
                                op=ALU.add)
                            t4 = attp.tile([128, 2 * NB * 4], BF16, tag="t4")
                            nc.vector.tensor_tensor(
                                out=t4[:].rearrange("m (y p) -> m y p", p=4),
                                in0=t8[:].rearrange("m (y p) -> m y p",
                                                    p=8)[:, :, 0:4],
                                in1=t8[:].rearrange("m (y p) -> m y p",
                                                    p=8)[:, :, 4:8],
                                op=ALU.add)
                            t2 = attp.tile([128, 2 * NB * 2], BF16, tag="t2")
                            nc.vector.tensor_tensor(
                                out=t2[:].rearrange("m (y p) -> m y p", p=2),
                                in0=t4[:].rearrange("m (y p) -> m y p",
                                                    p=4)[:, :, 0:2],
                                in1=t4[:].rearrange("m (y p) -> m y p",
                                                    p=4)[:, :, 2:4],
                                op=ALU.add)
                            ab4 = abtp.tile([128, 2 * NB], BF16, tag="ab")
                            nc.vector.tensor_tensor(
                                out=ab4[:].rearrange("m (y p) -> m y p", p=1),
                                in0=t2[:].rearrange("m (y p) -> m y p",
                                                    p=2)[:, :, 0:1],
                                in1=t2[:].rearrange("m (y p) -> m y p",
                                                    p=2)[:, :, 1:2],
                                op=ALU.add)
                            for kk in range(2):
                                attnT.append(
                                    ab4[:, kk * NB:(kk + 1) * NB])
                    nc.leave_named_scope(f"att{t}", sc_[0], False)

                    # (c) attn-part matmuls per quad: quad0 fully first, so
                    # quad0's LSTM math overlaps quad1's matmul rounds
                    sc2 = nc.enter_named_scope(f"ga{t}", False)
                    c_new = [blkp.tile([128, 128], F32, tag="c", bufs=4,
                                       name=f"cn{q}_{t}") for q in range(2)]
                    hbf = [blkp.tile([128, 128], BF16, tag="hbf",
                                     name=f"hbf{q}_{t}") for q in range(2)]
                    for q in range(2):
                        for k in range(KH, K2):
                            for j in range(4):
                                g = 4 * q + j
                                gsl = slice(g * 512, (g + 1) * 512)
                                gmm(psg[q], j, attnT[k - KH], w2[k][:, gsl],
                                    False, k == K2 - 1)
                        quad_math(q, psg, c_b, c_new, hbf, t)
                    nc.leave_named_scope(f"ga{t}", sc2[0], False)

                    # (d) next step: transpose each quad's h as it lands,
                    # U-init filler between, then scores + h-part split by
                    # which hT tile they need
                    sc4_ = nc.enter_named_scope(f"nxt{t}", False)
                    if not last:
                        psg2 = new_gate_psums(t + 1)
                        tr_ps = pst_p.tile([128, 256], BF16, tag="tr",
                                           name=f"tr{t}")
                        hT_new = []

                        def tr_quad(q):
                            nc.tensor.transpose(
                                tr_ps[:, q * 128:(q + 1) * 128],
                                hbf[q][:], id128[:])
                            ht_ = htp.tile([128, 128], BF16, tag="htq",
                                           name=f"ht{q}_{t}")
                            nc.vector.tensor_copy(
                                ht_[:], tr_ps[:, q * 128:(q + 1) * 128])
                            hT_new.append(ht_)

                        def hts_new(k):
                            q, gp = divmod(k, 4)
                            return hT_new[q][:, 32 * gp:32 * gp + 32]

                        tr_quad(0)
                        issue_umm(psg2, u_next)
                        tr_quad(1)
                        ps_s2 = pss_p.tile([128, NB * P], F32, tag="s",
                                           name=f"s{t + 1}")
                        issue_scores(ps_s2, hts_new, range(4))
                        issue_scores(ps_s2, hts_new, range(4, KH))
                        # masked extraction (DVE) runs while the first h-part
                        # rounds stream; the strip-sum matmul is slotted
                        # after 2 k-rounds so its input is ready on arrival
                        sc4_t = issue_masked(ps_s2)
                        issue_h_part(psg2, hts_new, range(2))
                        sc = issue_bcast(sc4_t, t + 1)
                        issue_h_part(psg2, hts_new, range(2, KH))
                        ps_s = ps_s2
                        psg = psg2
                        hTq = hT_new
                        c_b = c_new
                        u_t = u_next
                    nc.leave_named_scope(f"nxt{t}", sc4_[0], False)

            wh_pool_cm.__exit__(None, None, None)

    nc.compile()
    return nc


def prepare_inputs(x, A, Wx, Wh, Wattn, b, t_steps=T):
    """Host-side sharding + layout prep. Returns list of per-core input maps."""
    x = np.asarray(x, dtype=np.float32)
    A = np.asarray(A, dtype=np.float32)
    Wx = np.asarray(Wx, dtype=np.float32)
    Wh = np.asarray(Wh, dtype=np.float32)
    Wattn = np.asarray(Wattn, dtype=np.float32)
    b = np.asarray(b, dtype=np.float32)

    perm = _gate_perm()
    wx_p = np.ascontiguousarray(Wx[:, perm]).astype(BF)
    w2_p = np.ascontiguousarray(np.vstack([Wh, Wattn])[:, perm]).astype(BF)
    b128 = np.ascontiguousarray(
        np.broadcast_to(b[perm], (128, G))).astype(np.float32)
    mask32 = np.zeros((NB, NB * P), dtype=BF)
    for n in range(NB):
        mask32[n, n * P:(n + 1) * P] = 1
    mask = np.ascontiguousarray(np.tile(mask32, (4, 1)))
    ident = np.eye(NB, dtype=BF)
    id128 = np.eye(128, dtype=BF)
    ones = np.ones((1, 128), dtype=BF)
    sum4 = np.ascontiguousarray(
        np.tile(np.eye(NB, dtype=np.float32), (4, 1)))

    in_maps = []
    for c in range(NCORES):
        x_c = x[c * NB:(c + 1) * NB, :t_steps]          # (NB, t, D)
        xr = x_c.transpose(1, 0, 2).reshape(t_steps * NB, D)  # t-major rows
        xT = np.ascontiguousarray(xr.T).astype(BF)       # (D, t*NB)
        A_c = A[c * NB:(c + 1) * NB].reshape(NB, H, P)
        at_c = np.ascontiguousarray(
            A_c.transpose(1, 0, 2).reshape(H, NB * P)).astype(BF)
        h0 = A_c.mean(axis=2).astype(np.float32)         # (NB, H)
        # quad-stacked initial cell state: block g of quad q lives on
        # partitions 32*(g%4), columns = h dims within the block
        h0q = np.empty((2 * 128, 128), dtype=np.float32)
        for g in range(8):
            q, gp = divmod(g, 4)
            h0q[q * 128 + gp * 32:q * 128 + (gp + 1) * 32, :] = \
                h0[:, g * 128:(g + 1) * 128]
        # packed transposed h0: tile q, col 32*gp+n, row c
        h0t2 = np.ascontiguousarray(
            h0q.reshape(2, 128, 128).transpose(0, 2, 1)
            .reshape(2 * 128, 128)).astype(BF)
        in_maps.append({
            "xT": xT, "wx": wx_p, "w2": w2_p, "b128": b128,
            "at": at_c, "h0t2": h0t2, "h0q": h0q,
            "mask": mask, "ones": ones, "ident": ident, "id128": id128,
            "sum4": sum4,
        })
    return in_maps


def kernel(x, A, Wx, Wh, Wattn, b):
    from concourse.bass_utils import run_bass_kernel_spmd

    key = T
    if key not in _NC_CACHE:
        _NC_CACHE[key] = build_nc(T)
    nc = _NC_CACHE[key]

    in_maps = prepare_inputs(x, A, Wx, Wh, Wattn, b)
    trace = bool(int(os.environ.get("KERNEL_TRACE", "0")))
    res = run_bass_kernel_spmd(nc, in_maps, core_ids=list(range(NCORES)),
                               trace=trace)
    if res.exec_time_ns is not None:
        print(f"HW exec time: {res.exec_time_ns} ns")
        kernel.last_exec_time_ns = res.exec_time_ns
    out = np.concatenate([r["out"] for r in res.results], axis=0)
    return out.astype(np.float32)


kernel.last_exec_time_ns = None
